# revision 16
# baseline (speedup 1.0000x reference)
"""CARAFE (content-aware reassembly of features) Trainium2 Bass kernel.

Problem (hardcoded shapes):
  x       [8, 128, 64, 64] f32
  comp_w  [64, 128, 1, 1]   1x1 conv -> BN(train stats) -> SiLU
  enc_w   [100, 64, 3, 3]   3x3 conv -> BN(train stats)
  pixel_shuffle(2) -> softmax over 25 taps -> weighted 5x5 (dilation 2)
  reassembly of nearest-upsampled x. Output [8, 128, 128, 128] f32.

Sharding: data-parallel over batch, 1 image per core on 8 cores.
BN batch stats are made exact with two tiny AllReduces (sum & sumsq).

Key layout trick: with output pixel (y,x) = (2i+di, 2j+dj) and tap (dy,dx),
the reassembly source is x[c, i+dy-2, j+dx-2] -- independent of (di,dj).
So everything runs at low resolution with shifted views of a zero-padded x;
the pixel-shuffle and nearest-upsample are folded into access patterns.
"""

import sys

import numpy as np

sys.path.insert(0, "/opt/trn_rl_repo")

P = 128          # partitions / input channels
MID = 64         # compressed channels
NENC = 100       # encoder output channels = 25 taps * 4 subpixels
H = W = 64
PX = H * W       # 4096 low-res pixels per image
HP = H + 4       # zero-padded (pad=2) low-res frame for 5x5 dil-2 taps
H1 = H + 2       # zero-padded (pad=1) frame for the 3x3 conv
HM = 2 * H       # 128 upsampled
OUT = HM * HM    # 16384 output pixels per image
NB = 8           # batch / cores
NSTAT = NB * PX  # BN normalization count (N*H*W)
EPS = 1e-5
CHUNK = 512      # free-dim chunk = 8 low-res rows
NCHUNK = PX // CHUNK

_CACHE = {}


def _build_program():
    import concourse.bass as bass
    import concourse.mybir as mybir
    import concourse.tile as tile
    from concourse import bacc

    fp32 = mybir.dt.float32
    bf16 = mybir.dt.bfloat16
    Alu = mybir.AluOpType
    Act = mybir.ActivationFunctionType

    nc = bacc.Bacc(None, num_devices=NB)

    with tile.TileContext(nc) as tc:
        with tc.tile_pool(name="dram", bufs=1, space="DRAM") as dram:
            # I/O
            x_d = dram.tile([P, PX], fp32, kind="ExternalInput", name="x", uniquify=False)
            w1t_d = dram.tile([P, MID], fp32, kind="ExternalInput", name="w1t", uniquify=False)
            w2t_d = dram.tile([MID, 9 * NENC], fp32, kind="ExternalInput", name="w2t", uniquify=False)
            g1_d = dram.tile([MID, 2], fp32, kind="ExternalInput", name="g1b1", uniquify=False)
            g2_d = dram.tile([NENC, 2], fp32, kind="ExternalInput", name="g2b2", uniquify=False)
            sel4_d = dram.tile([NENC, 4], fp32, kind="ExternalInput", name="sel4", uniquify=False)
            sel100_d = dram.tile([4, NENC], fp32, kind="ExternalInput", name="sel100", uniquify=False)
            ones1_d = dram.tile([1, P], fp32, kind="ExternalInput", name="ones1", uniquify=False)
            eye_d = dram.tile([NENC, NENC], fp32, kind="ExternalInput", name="eye100", uniquify=False)
            out_d = dram.tile([P, OUT], fp32, kind="ExternalOutput", name="out", uniquify=False)
            # collective bounce buffers (internal DRAM)
            ar1_in = dram.tile([2, MID], fp32, name="ar1_in")
            ar1_out = dram.tile([2, MID], fp32, name="ar1_out")
            ar2_in = dram.tile([2, NENC], fp32, name="ar2_in")
            ar2_out = dram.tile([2, NENC], fp32, name="ar2_out")

            with (
                tc.tile_pool(name="const", bufs=1) as const,
                tc.tile_pool(name="big", bufs=1) as big,
                tc.tile_pool(name="small", bufs=1) as small,
                tc.tile_pool(name="scratch", bufs=3) as scratch,
                tc.tile_pool(name="tmpp", bufs=3) as tmpp,
                tc.tile_pool(name="ps", bufs=2, space="PSUM") as ps,
            ):
                # ---- load constants ----
                w1t = const.tile([P, MID], fp32)
                nc.sync.dma_start(w1t[:], w1t_d[:])
                w2t = const.tile([MID, 9 * NENC], fp32)
                nc.sync.dma_start(w2t[:], w2t_d[:])
                g1b1 = const.tile([MID, 2], fp32)
                nc.sync.dma_start(g1b1[:], g1_d[:])
                g2b2 = const.tile([NENC, 2], fp32)
                nc.sync.dma_start(g2b2[:], g2_d[:])
                sel4 = const.tile([NENC, 4], fp32)
                nc.sync.dma_start(sel4[:], sel4_d[:])
                sel100 = const.tile([4, NENC], fp32)
                nc.sync.dma_start(sel100[:], sel100_d[:])
                ones1 = const.tile([1, P], fp32)
                nc.sync.dma_start(ones1[:], ones1_d[:])
                eye100 = const.tile([NENC, NENC], fp32)
                nc.sync.dma_start(eye100[:], eye_d[:])

                # ---- padded x ----
                xpad = big.tile([P, HP, HP], fp32)
                nc.vector.memset(xpad[:], 0.0)
                nc.sync.dma_start(xpad[:, 2 : 2 + H, 2 : 2 + W], x_d[:].rearrange("p (h w) -> p h w", h=H))


                # ---- conv1 (1x1, 128->64) + stats ----
                y1 = big.tile([MID, PX], fp32, tag="ybuf", name="y1")
                s1c = small.tile([MID, NCHUNK], fp32)
                ss1c = small.tile([MID, NCHUNK], fp32)
                for c in range(NCHUNK):
                    r0 = c * 8
                    pt = ps.tile([P, 4 * CHUNK], fp32, tag="b", name="pt1")[:MID, :CHUNK]
                    nc.tensor.matmul(
                        pt[:], w1t[:], xpad[:, 2 + r0 : 10 + r0, 2 : 2 + W], start=True, stop=True
                    )
                    nc.scalar.activation(
                        y1[:, c * CHUNK : (c + 1) * CHUNK], pt[:], Act.Copy,
                        accum_out=s1c[:, c : c + 1],
                    )
                    sq = scratch.tile([MID, CHUNK], fp32, tag="sq1")
                    nc.scalar.activation(
                        sq[:], pt[:], Act.Square, accum_out=ss1c[:, c : c + 1]
                    )

                # ---- BN1 stats allreduce ----
                st1 = small.tile([MID, 2], fp32)
                nc.vector.tensor_reduce(st1[:, 0:1], s1c[:], mybir.AxisListType.X, Alu.add)
                nc.vector.tensor_reduce(st1[:, 1:2], ss1c[:], mybir.AxisListType.X, Alu.add)
                nc.sync.dma_start(ar1_in[:], st1[:])
                nc.gpsimd.collective_compute(
                    "AllReduce", Alu.add, replica_groups=[list(range(NB))],
                    ins=[ar1_in[:]], outs=[ar1_out[:]],
                )
                st1r = small.tile([MID, 2], fp32)
                nc.sync.dma_start(st1r[:], ar1_out[:])

                def bn_coeffs(pool, stats, gb, nchan, tag):
                    # stats [C,2] (sum, sumsq) -> scale/bias [C,1] each
                    m = pool.tile([nchan, 4], fp32, tag=tag)
                    nc.vector.tensor_scalar_mul(m[:, 0:1], stats[:, 0:1], 1.0 / NSTAT)
                    nc.vector.tensor_scalar_mul(m[:, 1:2], stats[:, 1:2], 1.0 / NSTAT)
                    nc.vector.tensor_tensor(m[:, 2:3], m[:, 0:1], m[:, 0:1], Alu.mult)
                    nc.vector.tensor_tensor(m[:, 3:4], m[:, 1:2], m[:, 2:3], Alu.subtract)
                    epst = pool.tile([nchan, 1], fp32, tag=tag + "e")
                    nc.vector.memset(epst[:], EPS)
                    std = pool.tile([nchan, 1], fp32, tag=tag + "s")
                    nc.scalar.activation(std[:], m[:, 3:4], Act.Sqrt, bias=epst[:])
                    inv = pool.tile([nchan, 1], fp32, tag=tag + "i")
                    nc.vector.reciprocal(inv[:], std[:])
                    sc = pool.tile([nchan, 2], fp32, tag=tag + "c")
                    # scale = gamma * inv ; bias = beta - mean*scale
                    nc.vector.tensor_tensor(sc[:, 0:1], gb[:, 0:1], inv[:], Alu.mult)
                    tmpm = pool.tile([nchan, 1], fp32, tag=tag + "m")
                    nc.vector.tensor_tensor(tmpm[:], m[:, 0:1], sc[:, 0:1], Alu.mult)
                    nc.vector.tensor_tensor(sc[:, 1:2], gb[:, 1:2], tmpm[:], Alu.subtract)
                    return sc

                sc1 = bn_coeffs(small, st1r, g1b1, MID, "bn1")

                # ---- BN1 + SiLU into padded t1 ----
                t1pad = big.tile([MID, H1, H1], fp32)
                nc.vector.memset(t1pad[:], 0.0)
                for c in range(NCHUNK):
                    r0 = c * 8
                    nc.scalar.activation(
                        t1pad[:, 1 + r0 : 9 + r0, 1 : 1 + W],
                        y1[:, c * CHUNK : (c + 1) * CHUNK],
                        Act.Silu, bias=sc1[:, 1:2], scale=sc1[:, 0:1],
                    )

                # ---- conv2 (3x3, 64->100) + stats ----
                y2 = big.tile([NENC, PX], fp32)
                s2c = small.tile([NENC, NCHUNK], fp32)
                ss2c = small.tile([NENC, NCHUNK], fp32)
                for c in range(NCHUNK):
                    r0 = c * 8
                    pt = ps.tile([P, 4 * CHUNK], fp32, tag="b", name="pt2")[:NENC, :CHUNK]
                    for tap in range(9):
                        dy, dx = tap // 3, tap % 3
                        nc.tensor.matmul(
                            pt[:],
                            w2t[:, tap * NENC : (tap + 1) * NENC],
                            t1pad[:, r0 + dy : r0 + dy + 8, dx : dx + W],
                            start=(tap == 0), stop=(tap == 8),
                        )
                    nc.scalar.activation(
                        y2[:, c * CHUNK : (c + 1) * CHUNK], pt[:], Act.Copy,
                        accum_out=s2c[:, c : c + 1],
                    )
                    sq = scratch.tile([NENC, CHUNK], fp32, tag="sq2")
                    nc.scalar.activation(
                        sq[:], pt[:], Act.Square, accum_out=ss2c[:, c : c + 1]
                    )

                # ---- BN2 stats allreduce ----
                st2 = small.tile([NENC, 2], fp32)
                nc.vector.tensor_reduce(st2[:, 0:1], s2c[:], mybir.AxisListType.X, Alu.add)
                nc.vector.tensor_reduce(st2[:, 1:2], ss2c[:], mybir.AxisListType.X, Alu.add)
                nc.sync.dma_start(ar2_in[:], st2[:])
                nc.gpsimd.collective_compute(
                    "AllReduce", Alu.add, replica_groups=[list(range(NB))],
                    ins=[ar2_in[:]], outs=[ar2_out[:]],
                )
                st2r = small.tile([NENC, 2], fp32)
                nc.sync.dma_start(st2r[:], ar2_out[:])
                sc2 = bn_coeffs(small, st2r, g2b2, NENC, "bn2")

                # ---- softmax numerators: e = exp(BN2(y2)) ----
                # BN output is ~N(0,1): exp without max-subtraction is safe in f32.
                esm = big.tile([NENC, PX], fp32, tag="ybuf", name="esm")
                nc.scalar.activation(esm[:], y2[:], Act.Exp, bias=sc2[:, 1:2], scale=sc2[:, 0:1])

                # ---- softmax denominators + normalized weights ----
                r4 = big.tile([4, PX], fp32)
                wsm = y2  # y2 fully consumed by esm; reuse its storage
                for c in range(NCHUNK):
                    sl = slice(c * CHUNK, (c + 1) * CHUNK)
                    pd = ps.tile([P, 4 * CHUNK], fp32, tag="b", name="pd")[:4, :CHUNK]
                    nc.tensor.matmul(pd[:], sel4[:], esm[:, sl], start=True, stop=True)
                    nc.vector.reciprocal(r4[:, sl], pd[:])
                for c in range(NCHUNK):
                    sl = slice(c * CHUNK, (c + 1) * CHUNK)
                    pr = ps.tile([P, 4 * CHUNK], fp32, tag="b", name="pr")[:NENC, :CHUNK]
                    nc.tensor.matmul(pr[:], sel100[:], r4[:, sl], start=True, stop=True)
                    nc.vector.tensor_tensor(wsm[:, sl], esm[:, sl], pr[:], Alu.mult)

                # ---- reassembly ----
                HALF = 4 * CHUNK  # 2048 low-res px = 32 low-res rows
                for h in range(2):
                    acch = tmpp.tile([P, HM // 2, HM], fp32, tag="acc", bufs=1, name="acch")
                    r0 = h * 32
                    for s in range(4):
                        di, dj = s // 2, s % 2
                        for k in range(25):
                            dy, dx = k // 5, k % 5
                            ch = k * 4 + s
                            wb = ps.tile([P, HALF], fp32, tag="b", name="wb")
                            onehot = eye100[:, ch : ch + 1].to_broadcast((NENC, P))
                            for c4 in range(4):
                                nc.tensor.matmul(
                                    wb[:, c4 * CHUNK : (c4 + 1) * CHUNK],
                                    onehot,
                                    wsm[:, (h * 4 + c4) * CHUNK : (h * 4 + c4 + 1) * CHUNK],
                                    start=True, stop=True,
                                )
                            xv = xpad[:, r0 + dy : r0 + dy + 32, dx : dx + W]
                            accv = acch[:, di : di + 63 : 2, dj : dj + 127 : 2]
                            if k == 0:
                                nc.vector.tensor_tensor(accv, wb[:], xv, Alu.mult)
                            else:
                                tm = tmpp.tile([P, HALF], fp32, tag="tm")
                                nc.vector.tensor_tensor(tm[:], wb[:], xv, Alu.mult)
                                nc.vector.tensor_tensor(accv, accv, tm[:], Alu.add)
                    nc.sync.dma_start(
                        out_d[:, h * (OUT // 2) : (h + 1) * (OUT // 2)],
                        acch[:].rearrange("p h w -> p (h w)"),
                    )

    nc.compile()
    return nc


def _prep_shared(comp_w, comp_g, comp_b, enc_w, enc_g, enc_b):
    w1t = np.ascontiguousarray(comp_w.reshape(MID, P).T)            # [128, 64]
    # w2t[tap] = enc_w[:, :, dy, dx].T  -> [64, 100] per tap, taps flattened
    w2t = np.ascontiguousarray(
        enc_w.transpose(2, 3, 1, 0).reshape(9, MID, NENC).transpose(1, 0, 2).reshape(MID, 9 * NENC)
    )
    g1b1 = np.stack([comp_g, comp_b], axis=1).astype(np.float32)    # [64, 2]
    g2b2 = np.stack([enc_g, enc_b], axis=1).astype(np.float32)      # [100, 2]
    ch = np.arange(NENC)
    sel4 = (ch[:, None] % 4 == np.arange(4)[None, :]).astype(np.float32)   # [100, 4]
    sel100 = np.ascontiguousarray(sel4.T)                                   # [4, 100]
    ones1 = np.ones((1, P), np.float32)
    eye100 = np.eye(NENC, dtype=np.float32)
    return dict(w1t=w1t, w2t=w2t, g1b1=g1b1, g2b2=g2b2, sel4=sel4, sel100=sel100, ones1=ones1, eye100=eye100)


def kernel(x, comp_w, comp_g, comp_b, enc_w, enc_g, enc_b):
    from concourse.bass_utils import run_bass_kernel_spmd

    x = np.asarray(x, np.float32)
    shared = _prep_shared(
        np.asarray(comp_w, np.float32), np.asarray(comp_g, np.float32),
        np.asarray(comp_b, np.float32), np.asarray(enc_w, np.float32),
        np.asarray(enc_g, np.float32), np.asarray(enc_b, np.float32),
    )
    if "nc" not in _CACHE:
        _CACHE["nc"] = _build_program()
    nc = _CACHE["nc"]

    in_maps = []
    for i in range(NB):
        m = dict(shared)
        m["x"] = np.ascontiguousarray(x[i].reshape(P, PX))
        in_maps.append(m)

    res = run_bass_kernel_spmd(nc, in_maps, list(range(NB)))
    out = np.stack([res.results[i]["out"].reshape(P, HM, HM) for i in range(NB)])
    return out.astype(np.float32)
